# revision 64
# baseline (speedup 1.0000x reference)
"""Trainium2 Bass kernel for AFT-style sparse attention (nn_AFTKVR).

Reference computation (per batch b):
    q,k,v = x @ W{q,k,v}.T          # [T=1024, D=256], H=4 heads x d=64
    ew = exp(wbias)                  # [T, K=63] neighbor weights
    ek = exp(k); kv = ek * v
    num[t] = sum_k ew[t,k] * kv[idx[t,k]]   (idx = row+col neighbors on 32x32 grid)
    den[t] = sum_k ew[t,k] * ek[idx[t,k]]
    out = sigmoid(q) * num / den

Sharding: 8 cores = 4 batches x 2 head-pairs (128 features each). No collectives.

Per-core device algorithm (all matmul operands bf16, fp32 PSUM accumulation):
  - q projected feature-major -> sigmoid(qT) [128f, 1024t]
  - k|v projected token-major per 128-token slice (lhsT = xT slice) -> ek, kv
  - k|v ALSO projected with grid-transposed token order (lhsT free AP permuted)
    -> ekp, kvp, giving contiguous lhsT tiles for the grid-column reduction
  - The 63-neighbor gather+reduce decomposes into block-diagonal matmuls:
      row part: out[f, rows-slice] += kv_slice.T @ Wrow_g   (contiguous PSUM cols)
      col part: out[f, strided cols] += kvp_slice.T @ Wcol_g (strided PSUM out AP,
                accumulated into the same PSUM via per-element accumulate bits)
    PSUM banks are pre-opened with a K=1 zero-matmul (start=True over the whole
    bank) so all real matmuls can accumulate in any order.
  - combine: w2 = sigmoid(qT) * numT * reciprocal(denT); DMA out feature-major;
    host transposes back to token-major during unshard.
"""

import os
from contextlib import ExitStack

import ml_dtypes
import numpy as np

import concourse.bass as bass
import concourse.tile as tile
from concourse import bacc, mybir
from concourse.bass_utils import run_bass_kernel_spmd

BF = mybir.dt.bfloat16
F32 = mybir.dt.float32
AF = mybir.ActivationFunctionType

N = 32          # grid side
T = N * N       # tokens
D = 256         # model dim
F = 128         # features per core (2 heads x 64)
NEG = -1e30     # exp(NEG) == 0

LAST_RESULT = None  # BassKernelResults of the most recent run (for profiling)
_CACHED_NC = None


def _build_nc():
    nc = bacc.Bacc("TRN2", target_bir_lowering=False, debug=False)

    xt_d = nc.declare_dram_parameter("xt", [128, 2048], BF, isOutput=False)
    xt2_d = nc.declare_dram_parameter("xt2", [128, 2048], BF, isOutput=False)
    wq_d = nc.declare_dram_parameter("wq", [128, 256], BF, isOutput=False)
    wkv_d = nc.declare_dram_parameter("wkv", [128, 512], BF, isOutput=False)
    wrow_d = nc.declare_dram_parameter("wrow", [128, 1024], BF, isOutput=False)
    wcol_d = nc.declare_dram_parameter("wcol", [128, 1024], BF, isOutput=False)
    out_d = nc.declare_dram_parameter("out", [128, 1024], BF, isOutput=True)

    from concourse.tile_rust import add_dep_helper

    with tile.TileContext(nc) as tc, ExitStack() as ctx:
        sb = ctx.enter_context(tc.tile_pool(name="sb", bufs=1))
        ps_q = ctx.enter_context(tc.tile_pool(name="ps_q", bufs=1, space="PSUM"))
        ps_kv = ctx.enter_context(tc.tile_pool(name="ps_kv", bufs=3, space="PSUM"))
        ps_g = ctx.enter_context(tc.tile_pool(name="ps_g", bufs=1, space="PSUM"))

        xt = sb.tile([128, 2048], BF, tag="xt")
        xt2 = sb.tile([128, 2048], BF, tag="xt2")
        wq = sb.tile([128, 256], BF, tag="wq")
        wkv = sb.tile([128, 512], BF, tag="wkv")
        wrow_e = sb.tile([128, 1024], BF, tag="wrow_e")
        wcol_e = sb.tile([128, 1024], BF, tag="wcol_e")
        sq = sb.tile([128, 1024], F32, tag="sq")
        ek = sb.tile([128, 1024], BF, tag="ek")
        kv = sb.tile([128, 1024], BF, tag="kv")
        ekp = sb.tile([128, 1024], BF, tag="ekp")
        kvp = sb.tile([128, 1024], BF, tag="kvp")
        rden = sb.tile([128, 1024], F32, tag="rden")
        w1 = sb.tile([128, 1024], F32, tag="w1")
        w2 = sb.tile([128, 1024], BF, tag="w2")
        warm = sb.tile([128, 512], BF, tag="warm")

        # input loads, in consumption-priority order; xt/xt2 stream in
        # pair-sized chunks (both d-halves of a 256-token span arrive
        # together) so the first k|v matmuls start as early as possible
        nc.sync.dma_start(out=wkv[:], in_=wkv_d[:])
        nc.sync.dma_start(out=wq[:], in_=wq_d[:])
        for dst, srcd in ((xt, xt_d), (xt2, xt2_d)):
            for qtr in range(4):
                for kh in range(2):
                    cs = slice(kh * 1024 + qtr * 256, kh * 1024 + qtr * 256 + 256)
                    nc.sync.dma_start(out=dst[:, cs], in_=srcd[:, cs])
        nc.sync.dma_start(out=wrow_e[:], in_=wrow_d[:])
        nc.sync.dma_start(out=wcol_e[:], in_=wcol_d[:])

        # PE warm-up: dummy matmuls while the input DMAs stream in, so the
        # HAM clock gate is released (1.2 -> 2.4 GHz) before the real work
        nc.gpsimd.memset(warm[:], 0.0)
        for i in range(10):
            wps = ps_kv.tile([128, 512], F32, tag="kvps")
            nc.tensor.matmul(wps[:], warm[:, 0:128], warm[:], start=True,
                             stop=True)

        # zero the grid accumulators on the (idle-early) DVE; every grid
        # matmul then accumulates with start=False + skip_group_check
        numT = ps_g.tile([128, 1024], F32, tag="numT")
        denT = ps_g.tile([128, 1024], F32, tag="denT")
        for t_ in (numT, denT):
            for bank in range(2):
                nc.vector.memset(t_[:, bank * 512:(bank + 1) * 512], 0.0)

        # q projection, feature-major: qT[o, t] accumulated over 2 d-halves,
        # copied to SBUF right away (frees the PSUM bank; DVE is idle early)
        qsb = sb.tile([128, 1024], F32, tag="qsb")
        for nh in range(2):
            qp = ps_q.tile([128, 512], F32, tag="qp")
            for kh in range(2):
                nc.tensor.matmul(
                    qp[:],
                    wq[:, kh * 128:(kh + 1) * 128],
                    xt[:, kh * 1024 + nh * 512: kh * 1024 + nh * 512 + 512],
                    start=(kh == 0), stop=(kh == 1),
                )
            nc.vector.tensor_copy(qsb[:, nh * 512:(nh + 1) * 512], qp[:])

        # sigmoids early (from SBUF): ACT does Sigmoid once, then only Exp
        nc.scalar.activation(sq[:, 0:512], qsb[:, 0:512], AF.Sigmoid)
        nc.scalar.activation(sq[:, 512:1024], qsb[:, 512:1024], AF.Sigmoid)

        # k|v projections token-major; normal token order (xt) feeds the row
        # part, grid-transposed token order (xt2) feeds the col part. Two
        # token-slices share one PSUM tile so exp/mul run at [2,128] each.
        ek_vw = ek[:].rearrange("p (g f) -> p g f", f=128)
        kv_vw = kv[:].rearrange("p (g f) -> p g f", f=128)
        ekp_vw = ekp[:].rearrange("p (g f) -> p g f", f=128)
        kvp_vw = kvp[:].rearrange("p (g f) -> p g f", f=128)
        for src, ek_t, kv_t in ((xt, ek_vw, kv_vw), (xt2, ekp_vw, kvp_vw)):
            for pr in range(4):
                kvps = ps_kv.tile([128, 512], F32, tag="kvps")
                mm = {}
                for g2 in range(2):
                    g = 2 * pr + g2
                    for kh in range(2):
                        lhsT = src[:, kh * 1024 + g * 128: kh * 1024 + (g + 1) * 128]
                        mm[g2, kh] = nc.tensor.matmul(
                            kvps[:, g2 * 256:(g2 + 1) * 256],
                            lhsT,
                            wkv[:, kh * 256:(kh + 1) * 256],
                            start=(g2 == 0 and kh == 0),
                            stop=(g2 == 1 and kh == 1),
                        )
                # keep PSUM zero-region state machine ordering legal: the
                # start=True matmul first, the stop=True matmul last
                add_dep_helper(mm[1, 0].ins, mm[0, 0].ins, reason="psum start first")
                add_dep_helper(mm[1, 1].ins, mm[0, 1].ins, reason="psum stop last")
                kvps_v = kvps[:].rearrange("p (g c) -> p g c", g=2)
                ps = slice(2 * pr, 2 * pr + 2)
                nc.scalar.activation(ek_t[:, ps, :], kvps_v[:, :, 0:128], AF.Exp)
                nc.vector.tensor_mul(kv_t[:, ps, :], ek_t[:, ps, :],
                                     kvps_v[:, :, 128:256])

        # grid reduction: row part (contiguous out cols per 4-grid-row slice)
        GK = dict(start=False, stop=False, skip_group_check=True)
        for g in range(8):
            gs = slice(g * 128, (g + 1) * 128)
            nc.tensor.matmul(numT[:, gs], kv[:, gs], wrow_e[:, gs], **GK)
            nc.tensor.matmul(denT[:, gs], ek[:, gs], wrow_e[:, gs], **GK)

        # col part: out cols strided (token r*32+c); bank-major order so the
        # bank-0 combine below can start while bank-1 matmuls still run
        numT_v = numT[:].rearrange("p (r c) -> p c r", c=N)
        denT_v = denT[:].rearrange("p (r c) -> p c r", c=N)
        wcol_v = wcol_e[:].rearrange("p (g cb r) -> p g cb r", g=8, cb=4)
        for hb in range(2):
            rh = slice(hb * 16, (hb + 1) * 16)
            # den first: the combine's reciprocal (the longest chain) can
            # start as soon as this bank's den matmuls finish
            for g in range(8):
                gs = slice(g * 128, (g + 1) * 128)
                nc.tensor.matmul(denT_v[:, 4 * g:4 * (g + 1), rh],
                                 ekp[:, gs], wcol_v[:, g, :, rh], **GK)
            for g in range(8):
                gs = slice(g * 128, (g + 1) * 128)
                nc.tensor.matmul(numT_v[:, 4 * g:4 * (g + 1), rh],
                                 kvp[:, gs], wcol_v[:, g, :, rh], **GK)

        # combine in quarters: the trailing quarter's chain (recip -> 2 muls
        # -> DMA) is all that sits past the last col matmul. The SBUF-only
        # final multiply runs on the idle Pool engine for the first three
        # quarters so DVE keeps feeding reciprocals.
        for qt in range(4):
            hs = slice(qt * 256, (qt + 1) * 256)
            nc.vector.reciprocal_approx_fast(rden[:, hs], denT[:, hs])
            nc.vector.tensor_mul(w1[:, hs], rden[:, hs], numT[:, hs])
            eng = nc.vector if qt == 3 else nc.gpsimd
            eng.tensor_mul(w2[:, hs], w1[:, hs], sq[:, hs])
            nc.sync.dma_start(out=out_d[:, hs], in_=w2[:, hs])

    nc.compile()
    return nc


def _get_nc():
    global _CACHED_NC
    if _CACHED_NC is None:
        _CACHED_NC = _build_nc()
    return _CACHED_NC


def _interleave_halves(a):
    """[256, M] -> [128, 2*M] with cols (half, m); partitions = dim%128."""
    return np.concatenate([a[0:128], a[128:256]], axis=1)


def make_shards(x, Wq, Wk, Wv, wbias):
    """Build the per-core input maps (host-side layout/sharding only)."""
    bf = ml_dtypes.bfloat16
    B = x.shape[0]

    # neighbor-weight reorganization: for token t=(r,c), sorted wbias cols are
    #   [0, r)   -> col-neighbor grid-row j = pos
    #   [r, r+N) -> row-neighbor grid-col j = pos - r
    #   [r+N, 2N-1) -> col-neighbor grid-row j = pos - (N - 1)
    t_idx = np.arange(T)
    r_of = t_idx // N
    Wr = np.empty((T, N), np.float32)
    Wc = np.full((T, N), NEG, np.float32)
    for t in range(T):
        r = int(r_of[t])
        Wr[t] = wbias[t, r:r + N]
        Wc[t, :r] = wbias[t, :r]
        Wc[t, r + 1:] = wbias[t, r + N:]

    # block-diagonal row weights (exp-folded; off-diagonal exp(NEG) == 0):
    #   wrow[rb*32+j, g*128 + rb*32 + c] = exp(Wr[(4g+rb)*32 + c, j])
    wrow = np.full((128, 1024), NEG, np.float32)
    wcol = np.full((128, 1024), NEG, np.float32)
    rb, j, c = np.meshgrid(np.arange(4), np.arange(N), np.arange(N),
                           indexing="ij")
    for g in range(8):
        wrow[rb * N + j, g * 128 + rb * N + c] = Wr[(4 * g + rb) * N + c, j]
        # col weights: wcol[cb*32+j, g*128 + cb*32 + r] = Wc[r*32 + 4g+cb, j]
        cb, jj, rr = rb, j, c
        wcol[cb * N + jj, g * 128 + cb * N + rr] = Wc[rr * N + 4 * g + cb, jj]
    wrow = np.exp(wrow).astype(bf)
    wcol = np.exp(wcol).astype(bf)

    xt_b = []
    xt2_b = []
    for b in range(B):
        xtb = np.ascontiguousarray(_interleave_halves(x[b].T)).astype(bf)
        xt_b.append(xtb)
        # grid-transposed token order: col kh*1024 + c*32 + r <- kh*1024 + r*32 + c
        x2 = xtb.reshape(128, 2, N, N).transpose(0, 1, 3, 2).reshape(128, 2048)
        xt2_b.append(np.ascontiguousarray(x2))

    in_maps = []
    for core in range(8):
        b, hp = core // 2, core % 2
        sl = slice(hp * 128, (hp + 1) * 128)
        wq_c = _interleave_halves(Wq[sl].T).astype(bf)                # [128,256]
        k_h = Wk[sl].T.reshape(2, 128, 128)
        v_h = Wv[sl].T.reshape(2, 128, 128)
        wkv_c = np.concatenate([k_h[0], v_h[0], k_h[1], v_h[1]],
                               axis=1).astype(bf)                      # [128,512]
        in_maps.append({
            "xt": xt_b[b],
            "xt2": xt2_b[b],
            "wq": np.ascontiguousarray(wq_c),
            "wkv": np.ascontiguousarray(wkv_c),
            "wrow": wrow,
            "wcol": wcol,
        })
    return in_maps


def kernel(x, Wq, Wk, Wv, wbias, key_indices=None, **_unused):
    global LAST_RESULT
    x = np.asarray(x, np.float32)
    Wq = np.asarray(Wq, np.float32)
    Wk = np.asarray(Wk, np.float32)
    Wv = np.asarray(Wv, np.float32)
    wbias = np.asarray(wbias, np.float32)

    nc = _get_nc()
    in_maps = make_shards(x, Wq, Wk, Wv, wbias)
    res = run_bass_kernel_spmd(nc, in_maps, core_ids=list(range(8)))
    LAST_RESULT = res

    B = x.shape[0]
    out = np.empty((B, T, D), np.float32)
    for core in range(8):
        b, hp = core // 2, core % 2
        out[b, :, hp * 128:(hp + 1) * 128] = \
            res.results[core]["out"].astype(np.float32).T
    return out


# revision 65
# speedup vs baseline: 1.0609x; 1.0609x over previous
"""Trainium2 Bass kernel for AFT-style sparse attention (nn_AFTKVR).

Reference computation (per batch b):
    q,k,v = x @ W{q,k,v}.T          # [T=1024, D=256], H=4 heads x d=64
    ew = exp(wbias)                  # [T, K=63] neighbor weights
    ek = exp(k); kv = ek * v
    num[t] = sum_k ew[t,k] * kv[idx[t,k]]   (idx = row+col neighbors on 32x32 grid)
    den[t] = sum_k ew[t,k] * ek[idx[t,k]]
    out = sigmoid(q) * num / den

Sharding: 8 cores = 4 batches x 2 head-pairs (128 features each). No collectives.

Per-core device algorithm (all matmul operands bf16, fp32 PSUM accumulation):
  - q projected feature-major -> sigmoid(qT) [128f, 1024t]
  - k|v projected token-major per 128-token slice (lhsT = xT slice) -> ek, kv
  - k|v ALSO projected with grid-transposed token order (lhsT free AP permuted)
    -> ekp, kvp, giving contiguous lhsT tiles for the grid-column reduction
  - The 63-neighbor gather+reduce decomposes into block-diagonal matmuls:
      row part: out[f, rows-slice] += kv_slice.T @ Wrow_g   (contiguous PSUM cols)
      col part: out[f, strided cols] += kvp_slice.T @ Wcol_g (strided PSUM out AP,
                accumulated into the same PSUM via per-element accumulate bits)
    PSUM banks are pre-opened with a K=1 zero-matmul (start=True over the whole
    bank) so all real matmuls can accumulate in any order.
  - combine: w2 = sigmoid(qT) * numT * reciprocal(denT); DMA out feature-major;
    host transposes back to token-major during unshard.
"""

import os
from contextlib import ExitStack

import ml_dtypes
import numpy as np

import concourse.bass as bass
import concourse.tile as tile
from concourse import bacc, mybir
from concourse.bass_utils import run_bass_kernel_spmd

BF = mybir.dt.bfloat16
F32 = mybir.dt.float32
AF = mybir.ActivationFunctionType

N = 32          # grid side
T = N * N       # tokens
D = 256         # model dim
F = 128         # features per core (2 heads x 64)
NEG = -1e30     # exp(NEG) == 0

LAST_RESULT = None  # BassKernelResults of the most recent run (for profiling)
_CACHED_NC = None


def _build_nc():
    nc = bacc.Bacc("TRN2", target_bir_lowering=False, debug=False)

    xt_d = nc.declare_dram_parameter("xt", [128, 2048], BF, isOutput=False)
    xt2_d = nc.declare_dram_parameter("xt2", [128, 2048], BF, isOutput=False)
    wq_d = nc.declare_dram_parameter("wq", [128, 256], BF, isOutput=False)
    wkv_d = nc.declare_dram_parameter("wkv", [128, 512], BF, isOutput=False)
    wrow_d = nc.declare_dram_parameter("wrow", [128, 1024], BF, isOutput=False)
    wcol_d = nc.declare_dram_parameter("wcol", [128, 1024], BF, isOutput=False)
    out_d = nc.declare_dram_parameter("out", [128, 1024], BF, isOutput=True)

    from concourse.tile_rust import add_dep_helper

    with tile.TileContext(nc) as tc, ExitStack() as ctx:
        sb = ctx.enter_context(tc.tile_pool(name="sb", bufs=1))
        ps_q = ctx.enter_context(tc.tile_pool(name="ps_q", bufs=1, space="PSUM"))
        ps_kv = ctx.enter_context(tc.tile_pool(name="ps_kv", bufs=3, space="PSUM"))
        ps_g = ctx.enter_context(tc.tile_pool(name="ps_g", bufs=1, space="PSUM"))

        xt = sb.tile([128, 2048], BF, tag="xt")
        xt2 = sb.tile([128, 2048], BF, tag="xt2")
        wq = sb.tile([128, 256], BF, tag="wq")
        wkv = sb.tile([128, 512], BF, tag="wkv")
        wrow_e = sb.tile([128, 1024], BF, tag="wrow_e")
        wcol_e = sb.tile([128, 1024], BF, tag="wcol_e")
        sq = sb.tile([128, 1024], F32, tag="sq")
        ek = sb.tile([128, 1024], BF, tag="ek")
        kv = sb.tile([128, 1024], BF, tag="kv")
        ekp = sb.tile([128, 1024], BF, tag="ekp")
        kvp = sb.tile([128, 1024], BF, tag="kvp")
        rden = sb.tile([128, 1024], F32, tag="rden")
        w1 = sb.tile([128, 1024], F32, tag="w1")
        w2 = sb.tile([128, 1024], BF, tag="w2")
        warm = sb.tile([128, 512], BF, tag="warm")

        # input loads, in consumption-priority order; xt/xt2 stream in
        # pair-sized chunks (both d-halves of a 256-token span arrive
        # together) so the first k|v matmuls start as early as possible
        nc.sync.dma_start(out=wkv[:], in_=wkv_d[:])
        nc.sync.dma_start(out=wq[:], in_=wq_d[:])
        for dst, srcd in ((xt, xt_d), (xt2, xt2_d)):
            for qtr in range(4):
                for kh in range(2):
                    cs = slice(kh * 1024 + qtr * 256, kh * 1024 + qtr * 256 + 256)
                    nc.sync.dma_start(out=dst[:, cs], in_=srcd[:, cs])
        nc.sync.dma_start(out=wrow_e[:], in_=wrow_d[:])
        nc.sync.dma_start(out=wcol_e[:], in_=wcol_d[:])

        # PE warm-up: dummy matmuls while the input DMAs stream in, so the
        # HAM clock gate is released (1.2 -> 2.4 GHz) before the real work
        nc.gpsimd.memset(warm[:], 0.0)
        for i in range(8):
            wps = ps_kv.tile([128, 512], F32, tag="kvps")
            nc.tensor.matmul(wps[:, 0:256], warm[:, 0:128], warm[:, 0:256],
                             start=True, stop=True)

        # zero the grid accumulators on the (idle-early) DVE; every grid
        # matmul then accumulates with start=False + skip_group_check
        numT = ps_g.tile([128, 1024], F32, tag="numT")
        denT = ps_g.tile([128, 1024], F32, tag="denT")
        for t_ in (numT, denT):
            for bank in range(2):
                nc.vector.memset(t_[:, bank * 512:(bank + 1) * 512], 0.0)

        # q projection, feature-major: qT[o, t] accumulated over 2 d-halves,
        # copied to SBUF right away (frees the PSUM bank; DVE is idle early)
        qsb = sb.tile([128, 1024], F32, tag="qsb")
        for nh in range(2):
            qp = ps_q.tile([128, 512], F32, tag="qp")
            for kh in range(2):
                nc.tensor.matmul(
                    qp[:],
                    wq[:, kh * 128:(kh + 1) * 128],
                    xt[:, kh * 1024 + nh * 512: kh * 1024 + nh * 512 + 512],
                    start=(kh == 0), stop=(kh == 1),
                )
            nc.vector.tensor_copy(qsb[:, nh * 512:(nh + 1) * 512], qp[:])

        # sigmoids early (from SBUF): ACT does Sigmoid once, then only Exp
        nc.scalar.activation(sq[:, 0:512], qsb[:, 0:512], AF.Sigmoid)
        nc.scalar.activation(sq[:, 512:1024], qsb[:, 512:1024], AF.Sigmoid)

        # k|v projections token-major; normal token order (xt) feeds the row
        # part, grid-transposed token order (xt2) feeds the col part. Two
        # token-slices share one PSUM tile so exp/mul run at [2,128] each.
        ek_vw = ek[:].rearrange("p (g f) -> p g f", f=128)
        kv_vw = kv[:].rearrange("p (g f) -> p g f", f=128)
        ekp_vw = ekp[:].rearrange("p (g f) -> p g f", f=128)
        kvp_vw = kvp[:].rearrange("p (g f) -> p g f", f=128)
        for src, ek_t, kv_t in ((xt, ek_vw, kv_vw), (xt2, ekp_vw, kvp_vw)):
            for pr in range(4):
                kvps = ps_kv.tile([128, 512], F32, tag="kvps")
                mm = {}
                for g2 in range(2):
                    g = 2 * pr + g2
                    for kh in range(2):
                        lhsT = src[:, kh * 1024 + g * 128: kh * 1024 + (g + 1) * 128]
                        mm[g2, kh] = nc.tensor.matmul(
                            kvps[:, g2 * 256:(g2 + 1) * 256],
                            lhsT,
                            wkv[:, kh * 256:(kh + 1) * 256],
                            start=(g2 == 0 and kh == 0),
                            stop=(g2 == 1 and kh == 1),
                        )
                # keep PSUM zero-region state machine ordering legal: the
                # start=True matmul first, the stop=True matmul last
                add_dep_helper(mm[1, 0].ins, mm[0, 0].ins, reason="psum start first")
                add_dep_helper(mm[1, 1].ins, mm[0, 1].ins, reason="psum stop last")
                kvps_v = kvps[:].rearrange("p (g c) -> p g c", g=2)
                ps = slice(2 * pr, 2 * pr + 2)
                nc.scalar.activation(ek_t[:, ps, :], kvps_v[:, :, 0:128], AF.Exp)
                nc.vector.tensor_mul(kv_t[:, ps, :], ek_t[:, ps, :],
                                     kvps_v[:, :, 128:256])

        # grid reduction: row part (contiguous out cols per 4-grid-row slice)
        GK = dict(start=False, stop=False, skip_group_check=True)
        for g in range(8):
            gs = slice(g * 128, (g + 1) * 128)
            nc.tensor.matmul(numT[:, gs], kv[:, gs], wrow_e[:, gs], **GK)
            nc.tensor.matmul(denT[:, gs], ek[:, gs], wrow_e[:, gs], **GK)

        # col part: out cols strided (token r*32+c); bank-major order so the
        # bank-0 combine below can start while bank-1 matmuls still run
        numT_v = numT[:].rearrange("p (r c) -> p c r", c=N)
        denT_v = denT[:].rearrange("p (r c) -> p c r", c=N)
        wcol_v = wcol_e[:].rearrange("p (g cb r) -> p g cb r", g=8, cb=4)
        for hb in range(2):
            rh = slice(hb * 16, (hb + 1) * 16)
            # den first: the combine's reciprocal (the longest chain) can
            # start as soon as this bank's den matmuls finish
            for g in range(8):
                gs = slice(g * 128, (g + 1) * 128)
                nc.tensor.matmul(denT_v[:, 4 * g:4 * (g + 1), rh],
                                 ekp[:, gs], wcol_v[:, g, :, rh], **GK)
            for g in range(8):
                gs = slice(g * 128, (g + 1) * 128)
                nc.tensor.matmul(numT_v[:, 4 * g:4 * (g + 1), rh],
                                 kvp[:, gs], wcol_v[:, g, :, rh], **GK)

        # combine in quarters: the trailing quarter's chain (recip -> 2 muls
        # -> DMA) is all that sits past the last col matmul. The SBUF-only
        # final multiply runs on the idle Pool engine for the first three
        # quarters so DVE keeps feeding reciprocals.
        for qt in range(4):
            hs = slice(qt * 256, (qt + 1) * 256)
            nc.vector.reciprocal_approx_fast(rden[:, hs], denT[:, hs])
            nc.vector.tensor_mul(w1[:, hs], rden[:, hs], numT[:, hs])
            eng = nc.vector if qt == 3 else nc.gpsimd
            eng.tensor_mul(w2[:, hs], w1[:, hs], sq[:, hs])
            nc.sync.dma_start(out=out_d[:, hs], in_=w2[:, hs])

    nc.compile()
    return nc


def _get_nc():
    global _CACHED_NC
    if _CACHED_NC is None:
        _CACHED_NC = _build_nc()
    return _CACHED_NC


def _interleave_halves(a):
    """[256, M] -> [128, 2*M] with cols (half, m); partitions = dim%128."""
    return np.concatenate([a[0:128], a[128:256]], axis=1)


def make_shards(x, Wq, Wk, Wv, wbias):
    """Build the per-core input maps (host-side layout/sharding only)."""
    bf = ml_dtypes.bfloat16
    B = x.shape[0]

    # neighbor-weight reorganization: for token t=(r,c), sorted wbias cols are
    #   [0, r)   -> col-neighbor grid-row j = pos
    #   [r, r+N) -> row-neighbor grid-col j = pos - r
    #   [r+N, 2N-1) -> col-neighbor grid-row j = pos - (N - 1)
    t_idx = np.arange(T)
    r_of = t_idx // N
    Wr = np.empty((T, N), np.float32)
    Wc = np.full((T, N), NEG, np.float32)
    for t in range(T):
        r = int(r_of[t])
        Wr[t] = wbias[t, r:r + N]
        Wc[t, :r] = wbias[t, :r]
        Wc[t, r + 1:] = wbias[t, r + N:]

    # block-diagonal row weights (exp-folded; off-diagonal exp(NEG) == 0):
    #   wrow[rb*32+j, g*128 + rb*32 + c] = exp(Wr[(4g+rb)*32 + c, j])
    wrow = np.full((128, 1024), NEG, np.float32)
    wcol = np.full((128, 1024), NEG, np.float32)
    rb, j, c = np.meshgrid(np.arange(4), np.arange(N), np.arange(N),
                           indexing="ij")
    for g in range(8):
        wrow[rb * N + j, g * 128 + rb * N + c] = Wr[(4 * g + rb) * N + c, j]
        # col weights: wcol[cb*32+j, g*128 + cb*32 + r] = Wc[r*32 + 4g+cb, j]
        cb, jj, rr = rb, j, c
        wcol[cb * N + jj, g * 128 + cb * N + rr] = Wc[rr * N + 4 * g + cb, jj]
    wrow = np.exp(wrow).astype(bf)
    wcol = np.exp(wcol).astype(bf)

    xt_b = []
    xt2_b = []
    for b in range(B):
        xtb = np.ascontiguousarray(_interleave_halves(x[b].T)).astype(bf)
        xt_b.append(xtb)
        # grid-transposed token order: col kh*1024 + c*32 + r <- kh*1024 + r*32 + c
        x2 = xtb.reshape(128, 2, N, N).transpose(0, 1, 3, 2).reshape(128, 2048)
        xt2_b.append(np.ascontiguousarray(x2))

    in_maps = []
    for core in range(8):
        b, hp = core // 2, core % 2
        sl = slice(hp * 128, (hp + 1) * 128)
        wq_c = _interleave_halves(Wq[sl].T).astype(bf)                # [128,256]
        k_h = Wk[sl].T.reshape(2, 128, 128)
        v_h = Wv[sl].T.reshape(2, 128, 128)
        wkv_c = np.concatenate([k_h[0], v_h[0], k_h[1], v_h[1]],
                               axis=1).astype(bf)                      # [128,512]
        in_maps.append({
            "xt": xt_b[b],
            "xt2": xt2_b[b],
            "wq": np.ascontiguousarray(wq_c),
            "wkv": np.ascontiguousarray(wkv_c),
            "wrow": wrow,
            "wcol": wcol,
        })
    return in_maps


def kernel(x, Wq, Wk, Wv, wbias, key_indices=None, **_unused):
    global LAST_RESULT
    x = np.asarray(x, np.float32)
    Wq = np.asarray(Wq, np.float32)
    Wk = np.asarray(Wk, np.float32)
    Wv = np.asarray(Wv, np.float32)
    wbias = np.asarray(wbias, np.float32)

    nc = _get_nc()
    in_maps = make_shards(x, Wq, Wk, Wv, wbias)
    res = run_bass_kernel_spmd(nc, in_maps, core_ids=list(range(8)))
    LAST_RESULT = res

    B = x.shape[0]
    out = np.empty((B, T, D), np.float32)
    for core in range(8):
        b, hp = core // 2, core % 2
        out[b, :, hp * 128:(hp + 1) * 128] = \
            res.results[core]["out"].astype(np.float32).T
    return out


# revision 67
# speedup vs baseline: 1.1272x; 1.0625x over previous
"""Trainium2 Bass kernel for AFT-style sparse attention (nn_AFTKVR).

Reference computation (per batch b):
    q,k,v = x @ W{q,k,v}.T          # [T=1024, D=256], H=4 heads x d=64
    ew = exp(wbias)                  # [T, K=63] neighbor weights
    ek = exp(k); kv = ek * v
    num[t] = sum_k ew[t,k] * kv[idx[t,k]]   (idx = row+col neighbors on 32x32 grid)
    den[t] = sum_k ew[t,k] * ek[idx[t,k]]
    out = sigmoid(q) * num / den

Sharding: 8 cores = 4 batches x 2 head-pairs (128 features each). No collectives.

Per-core device algorithm (all matmul operands bf16, fp32 PSUM accumulation):
  - q projected feature-major -> sigmoid(qT) [128f, 1024t]
  - k|v projected token-major per 128-token slice (lhsT = xT slice) -> ek, kv
  - k|v ALSO projected with grid-transposed token order (lhsT free AP permuted)
    -> ekp, kvp, giving contiguous lhsT tiles for the grid-column reduction
  - The 63-neighbor gather+reduce decomposes into block-diagonal matmuls:
      row part: out[f, rows-slice] += kv_slice.T @ Wrow_g   (contiguous PSUM cols)
      col part: out[f, strided cols] += kvp_slice.T @ Wcol_g (strided PSUM out AP,
                accumulated into the same PSUM via per-element accumulate bits)
    PSUM banks are pre-opened with a K=1 zero-matmul (start=True over the whole
    bank) so all real matmuls can accumulate in any order.
  - combine: w2 = sigmoid(qT) * numT * reciprocal(denT); DMA out feature-major;
    host transposes back to token-major during unshard.
"""

import os
from contextlib import ExitStack

import ml_dtypes
import numpy as np

import concourse.bass as bass
import concourse.tile as tile
from concourse import bacc, mybir
from concourse.bass_utils import run_bass_kernel_spmd

BF = mybir.dt.bfloat16
F32 = mybir.dt.float32
AF = mybir.ActivationFunctionType

N = 32          # grid side
T = N * N       # tokens
D = 256         # model dim
F = 128         # features per core (2 heads x 64)
NEG = -1e30     # exp(NEG) == 0

LAST_RESULT = None  # BassKernelResults of the most recent run (for profiling)
_CACHED_NC = None


def _build_nc():
    nc = bacc.Bacc("TRN2", target_bir_lowering=False, debug=False)

    xt_d = nc.declare_dram_parameter("xt", [128, 2048], BF, isOutput=False)
    xt2_d = nc.declare_dram_parameter("xt2", [128, 2048], BF, isOutput=False)
    wq_d = nc.declare_dram_parameter("wq", [128, 256], BF, isOutput=False)
    wkv_d = nc.declare_dram_parameter("wkv", [128, 512], BF, isOutput=False)
    wrow_d = nc.declare_dram_parameter("wrow", [128, 1024], BF, isOutput=False)
    wcol_d = nc.declare_dram_parameter("wcol", [128, 1024], BF, isOutput=False)
    out_d = nc.declare_dram_parameter("out", [128, 1024], BF, isOutput=True)

    from concourse.tile_rust import add_dep_helper

    with tile.TileContext(nc) as tc, ExitStack() as ctx:
        sb = ctx.enter_context(tc.tile_pool(name="sb", bufs=1))
        ps_q = ctx.enter_context(tc.tile_pool(name="ps_q", bufs=1, space="PSUM"))
        ps_kv = ctx.enter_context(tc.tile_pool(name="ps_kv", bufs=3, space="PSUM"))
        ps_g = ctx.enter_context(tc.tile_pool(name="ps_g", bufs=1, space="PSUM"))

        xt = sb.tile([128, 2048], BF, tag="xt")
        xt2 = sb.tile([128, 2048], BF, tag="xt2")
        wq = sb.tile([128, 256], BF, tag="wq")
        wkv = sb.tile([128, 512], BF, tag="wkv")
        wrow_e = sb.tile([128, 1024], BF, tag="wrow_e")
        wcol_e = sb.tile([128, 1024], BF, tag="wcol_e")
        sq = sb.tile([128, 1024], F32, tag="sq")
        ek = sb.tile([128, 1024], BF, tag="ek")
        kv = sb.tile([128, 1024], BF, tag="kv")
        ekp = sb.tile([128, 1024], BF, tag="ekp")
        kvp = sb.tile([128, 1024], BF, tag="kvp")
        rden = sb.tile([128, 1024], F32, tag="rden")
        w1 = sb.tile([128, 1024], F32, tag="w1")
        w2 = sb.tile([128, 1024], BF, tag="w2")
        warm = sb.tile([128, 512], BF, tag="warm")

        # input loads, in consumption-priority order
        nc.sync.dma_start(out=xt[:, 0:1024], in_=xt_d[:, 0:1024])
        nc.sync.dma_start(out=xt[:, 1024:2048], in_=xt_d[:, 1024:2048])
        nc.sync.dma_start(out=wkv[:], in_=wkv_d[:])
        nc.sync.dma_start(out=wq[:], in_=wq_d[:])
        nc.sync.dma_start(out=xt2[:, 0:1024], in_=xt2_d[:, 0:1024])
        nc.sync.dma_start(out=xt2[:, 1024:2048], in_=xt2_d[:, 1024:2048])
        nc.sync.dma_start(out=wrow_e[:], in_=wrow_d[:])
        nc.sync.dma_start(out=wcol_e[:], in_=wcol_d[:])

        # PE warm-up: dummy matmuls while the input DMAs stream in, so the
        # HAM clock gate is released (1.2 -> 2.4 GHz) before the real work
        nc.gpsimd.memset(warm[:], 0.0)
        for i in range(10):
            wps = ps_kv.tile([128, 512], F32, tag="kvps")
            nc.tensor.matmul(wps[:], warm[:, 0:128], warm[:], start=True,
                             stop=True)

        # zero the grid accumulators on the (idle-early) DVE; every grid
        # matmul then accumulates with start=False + skip_group_check
        numT = ps_g.tile([128, 1024], F32, tag="numT")
        denT = ps_g.tile([128, 1024], F32, tag="denT")
        for t_ in (numT, denT):
            for bank in range(2):
                nc.vector.memset(t_[:, bank * 512:(bank + 1) * 512], 0.0)

        # q projection, feature-major: qT[o, t] accumulated over 2 d-halves,
        # copied to SBUF right away (frees the PSUM bank; DVE is idle early)
        qsb = sb.tile([128, 1024], F32, tag="qsb")
        for nh in range(2):
            qp = ps_q.tile([128, 512], F32, tag="qp")
            for kh in range(2):
                nc.tensor.matmul(
                    qp[:],
                    wq[:, kh * 128:(kh + 1) * 128],
                    xt[:, kh * 1024 + nh * 512: kh * 1024 + nh * 512 + 512],
                    start=(kh == 0), stop=(kh == 1),
                )
            nc.vector.tensor_copy(qsb[:, nh * 512:(nh + 1) * 512], qp[:])

        # sigmoids early (from SBUF): ACT does Sigmoid once, then only Exp
        nc.scalar.activation(sq[:, 0:512], qsb[:, 0:512], AF.Sigmoid)
        nc.scalar.activation(sq[:, 512:1024], qsb[:, 512:1024], AF.Sigmoid)

        # k|v projections token-major; normal token order (xt) feeds the row
        # part, grid-transposed token order (xt2) feeds the col part. Two
        # token-slices share one PSUM tile so exp/mul run at [2,128] each.
        ek_vw = ek[:].rearrange("p (g f) -> p g f", f=128)
        kv_vw = kv[:].rearrange("p (g f) -> p g f", f=128)
        ekp_vw = ekp[:].rearrange("p (g f) -> p g f", f=128)
        kvp_vw = kvp[:].rearrange("p (g f) -> p g f", f=128)
        for src, ek_t, kv_t in ((xt, ek_vw, kv_vw), (xt2, ekp_vw, kvp_vw)):
            for pr in range(4):
                kvps = ps_kv.tile([128, 512], F32, tag="kvps")
                mm = {}
                for g2 in range(2):
                    g = 2 * pr + g2
                    for kh in range(2):
                        lhsT = src[:, kh * 1024 + g * 128: kh * 1024 + (g + 1) * 128]
                        mm[g2, kh] = nc.tensor.matmul(
                            kvps[:, g2 * 256:(g2 + 1) * 256],
                            lhsT,
                            wkv[:, kh * 256:(kh + 1) * 256],
                            start=(g2 == 0 and kh == 0),
                            stop=(g2 == 1 and kh == 1),
                        )
                # keep PSUM zero-region state machine ordering legal: the
                # start=True matmul first, the stop=True matmul last
                add_dep_helper(mm[1, 0].ins, mm[0, 0].ins, reason="psum start first")
                add_dep_helper(mm[1, 1].ins, mm[0, 1].ins, reason="psum stop last")
                kvps_v = kvps[:].rearrange("p (g c) -> p g c", g=2)
                ps = slice(2 * pr, 2 * pr + 2)
                nc.scalar.activation(ek_t[:, ps, :], kvps_v[:, :, 0:128], AF.Exp)
                nc.vector.tensor_mul(kv_t[:, ps, :], ek_t[:, ps, :],
                                     kvps_v[:, :, 128:256])

        # grid reduction: row part (contiguous out cols per 4-grid-row slice)
        GK = dict(start=False, stop=False, skip_group_check=True)
        for g in range(8):
            gs = slice(g * 128, (g + 1) * 128)
            nc.tensor.matmul(numT[:, gs], kv[:, gs], wrow_e[:, gs], **GK)
            nc.tensor.matmul(denT[:, gs], ek[:, gs], wrow_e[:, gs], **GK)

        # col part: out cols strided (token r*32+c); bank-major order so the
        # bank-0 combine below can start while bank-1 matmuls still run
        numT_v = numT[:].rearrange("p (r c) -> p c r", c=N)
        denT_v = denT[:].rearrange("p (r c) -> p c r", c=N)
        wcol_v = wcol_e[:].rearrange("p (g cb r) -> p g cb r", g=8, cb=4)
        for hb in range(2):
            rh = slice(hb * 16, (hb + 1) * 16)
            # den first: the combine's reciprocal (the longest chain) can
            # start as soon as this bank's den matmuls finish
            for g in range(8):
                gs = slice(g * 128, (g + 1) * 128)
                nc.tensor.matmul(denT_v[:, 4 * g:4 * (g + 1), rh],
                                 ekp[:, gs], wcol_v[:, g, :, rh], **GK)
            for g in range(8):
                gs = slice(g * 128, (g + 1) * 128)
                nc.tensor.matmul(numT_v[:, 4 * g:4 * (g + 1), rh],
                                 kvp[:, gs], wcol_v[:, g, :, rh], **GK)

        # combine in quarters: the trailing quarter's chain (recip -> 2 muls
        # -> DMA) is all that sits past the last col matmul. The SBUF-only
        # final multiply runs on the idle Pool engine for the first three
        # quarters so DVE keeps feeding reciprocals.
        for qt in range(4):
            hs = slice(qt * 256, (qt + 1) * 256)
            nc.vector.reciprocal_approx_fast(rden[:, hs], denT[:, hs])
            nc.vector.tensor_mul(w1[:, hs], rden[:, hs], numT[:, hs])
            eng = nc.vector if qt == 3 else nc.gpsimd
            eng.tensor_mul(w2[:, hs], w1[:, hs], sq[:, hs])
            nc.sync.dma_start(out=out_d[:, hs], in_=w2[:, hs])

    nc.compile()
    return nc


def _get_nc():
    global _CACHED_NC
    if _CACHED_NC is None:
        _CACHED_NC = _build_nc()
    return _CACHED_NC


def _interleave_halves(a):
    """[256, M] -> [128, 2*M] with cols (half, m); partitions = dim%128."""
    return np.concatenate([a[0:128], a[128:256]], axis=1)


def make_shards(x, Wq, Wk, Wv, wbias):
    """Build the per-core input maps (host-side layout/sharding only)."""
    bf = ml_dtypes.bfloat16
    B = x.shape[0]

    # neighbor-weight reorganization: for token t=(r,c), sorted wbias cols are
    #   [0, r)   -> col-neighbor grid-row j = pos
    #   [r, r+N) -> row-neighbor grid-col j = pos - r
    #   [r+N, 2N-1) -> col-neighbor grid-row j = pos - (N - 1)
    t_idx = np.arange(T)
    r_of = t_idx // N
    Wr = np.empty((T, N), np.float32)
    Wc = np.full((T, N), NEG, np.float32)
    for t in range(T):
        r = int(r_of[t])
        Wr[t] = wbias[t, r:r + N]
        Wc[t, :r] = wbias[t, :r]
        Wc[t, r + 1:] = wbias[t, r + N:]

    # block-diagonal row weights (exp-folded; off-diagonal exp(NEG) == 0):
    #   wrow[rb*32+j, g*128 + rb*32 + c] = exp(Wr[(4g+rb)*32 + c, j])
    wrow = np.full((128, 1024), NEG, np.float32)
    wcol = np.full((128, 1024), NEG, np.float32)
    rb, j, c = np.meshgrid(np.arange(4), np.arange(N), np.arange(N),
                           indexing="ij")
    for g in range(8):
        wrow[rb * N + j, g * 128 + rb * N + c] = Wr[(4 * g + rb) * N + c, j]
        # col weights: wcol[cb*32+j, g*128 + cb*32 + r] = Wc[r*32 + 4g+cb, j]
        cb, jj, rr = rb, j, c
        wcol[cb * N + jj, g * 128 + cb * N + rr] = Wc[rr * N + 4 * g + cb, jj]
    wrow = np.exp(wrow).astype(bf)
    wcol = np.exp(wcol).astype(bf)

    xt_b = []
    xt2_b = []
    for b in range(B):
        xtb = np.ascontiguousarray(_interleave_halves(x[b].T)).astype(bf)
        xt_b.append(xtb)
        # grid-transposed token order: col kh*1024 + c*32 + r <- kh*1024 + r*32 + c
        x2 = xtb.reshape(128, 2, N, N).transpose(0, 1, 3, 2).reshape(128, 2048)
        xt2_b.append(np.ascontiguousarray(x2))

    in_maps = []
    for core in range(8):
        b, hp = core // 2, core % 2
        sl = slice(hp * 128, (hp + 1) * 128)
        wq_c = _interleave_halves(Wq[sl].T).astype(bf)                # [128,256]
        k_h = Wk[sl].T.reshape(2, 128, 128)
        v_h = Wv[sl].T.reshape(2, 128, 128)
        wkv_c = np.concatenate([k_h[0], v_h[0], k_h[1], v_h[1]],
                               axis=1).astype(bf)                      # [128,512]
        in_maps.append({
            "xt": xt_b[b],
            "xt2": xt2_b[b],
            "wq": np.ascontiguousarray(wq_c),
            "wkv": np.ascontiguousarray(wkv_c),
            "wrow": wrow,
            "wcol": wcol,
        })
    return in_maps


def kernel(x, Wq, Wk, Wv, wbias, key_indices=None, **_unused):
    global LAST_RESULT
    x = np.asarray(x, np.float32)
    Wq = np.asarray(Wq, np.float32)
    Wk = np.asarray(Wk, np.float32)
    Wv = np.asarray(Wv, np.float32)
    wbias = np.asarray(wbias, np.float32)

    nc = _get_nc()
    in_maps = make_shards(x, Wq, Wk, Wv, wbias)
    res = run_bass_kernel_spmd(nc, in_maps, core_ids=list(range(8)))
    LAST_RESULT = res

    B = x.shape[0]
    out = np.empty((B, T, D), np.float32)
    for core in range(8):
        b, hp = core // 2, core % 2
        out[b, :, hp * 128:(hp + 1) * 128] = \
            res.results[core]["out"].astype(np.float32).T
    return out


# revision 69
# speedup vs baseline: 1.1930x; 1.0583x over previous
"""Trainium2 Bass kernel for AFT-style sparse attention (nn_AFTKVR).

Reference computation (per batch b):
    q,k,v = x @ W{q,k,v}.T          # [T=1024, D=256], H=4 heads x d=64
    ew = exp(wbias)                  # [T, K=63] neighbor weights
    ek = exp(k); kv = ek * v
    num[t] = sum_k ew[t,k] * kv[idx[t,k]]   (idx = row+col neighbors on 32x32 grid)
    den[t] = sum_k ew[t,k] * ek[idx[t,k]]
    out = sigmoid(q) * num / den

Sharding: 8 cores = 4 batches x 2 head-pairs (128 features each). No collectives.

Per-core device algorithm (all matmul operands bf16, fp32 PSUM accumulation):
  - q projected feature-major -> sigmoid(qT) [128f, 1024t]
  - k|v projected token-major per 128-token slice (lhsT = xT slice) -> ek, kv
  - k|v ALSO projected from xt2 (host-prepared grid-transposed column order)
    -> ekp, kvp, giving contiguous lhsT tiles for the grid-column reduction
  - The 63-neighbor gather+reduce decomposes into block-diagonal matmuls:
      row part: out[f, rows-slice] += kv_slice.T @ Wrow_g   (contiguous PSUM cols)
      col part: out[f, strided cols] += kvp_slice.T @ Wcol_g (strided PSUM out AP,
                accumulated into the same PSUM via per-element accumulate bits)
    The grid PSUM tiles are zeroed by DVE memsets and every grid matmul runs
    with start=False + skip_group_check, so they can accumulate in any order.
  - combine (quartered): w2 = sigmoid(qT) * numT * recip_approx(denT); out is
    written feature-major bf16; host casts/transposes during unshard.
  - dummy K=1 matmuls warm the PE HAM clock gate while inputs stream in.
"""

import os
from contextlib import ExitStack

import ml_dtypes
import numpy as np

import concourse.bass as bass
import concourse.tile as tile
from concourse import bacc, mybir
from concourse.bass_utils import run_bass_kernel_spmd

BF = mybir.dt.bfloat16
F32 = mybir.dt.float32
AF = mybir.ActivationFunctionType

N = 32          # grid side
T = N * N       # tokens
D = 256         # model dim
F = 128         # features per core (2 heads x 64)
NEG = -1e30     # exp(NEG) == 0

LAST_RESULT = None  # BassKernelResults of the most recent run (for profiling)
_CACHED_NC = None


def _build_nc():
    nc = bacc.Bacc("TRN2", target_bir_lowering=False, debug=False)

    xt_d = nc.declare_dram_parameter("xt", [128, 2048], BF, isOutput=False)
    xt2_d = nc.declare_dram_parameter("xt2", [128, 2048], BF, isOutput=False)
    wq_d = nc.declare_dram_parameter("wq", [128, 256], BF, isOutput=False)
    wkv_d = nc.declare_dram_parameter("wkv", [128, 512], BF, isOutput=False)
    wrow_d = nc.declare_dram_parameter("wrow", [128, 1024], BF, isOutput=False)
    wcol_d = nc.declare_dram_parameter("wcol", [128, 1024], BF, isOutput=False)
    out_d = nc.declare_dram_parameter("out", [128, 1024], BF, isOutput=True)

    from concourse.tile_rust import add_dep_helper

    with tile.TileContext(nc) as tc, ExitStack() as ctx:
        sb = ctx.enter_context(tc.tile_pool(name="sb", bufs=1))
        ps_q = ctx.enter_context(tc.tile_pool(name="ps_q", bufs=1, space="PSUM"))
        ps_kv = ctx.enter_context(tc.tile_pool(name="ps_kv", bufs=3, space="PSUM"))
        ps_g = ctx.enter_context(tc.tile_pool(name="ps_g", bufs=1, space="PSUM"))

        xt = sb.tile([128, 2048], BF, tag="xt")
        xt2 = sb.tile([128, 2048], BF, tag="xt2")
        wq = sb.tile([128, 256], BF, tag="wq")
        wkv = sb.tile([128, 512], BF, tag="wkv")
        wrow_e = sb.tile([128, 1024], BF, tag="wrow_e")
        wcol_e = sb.tile([128, 1024], BF, tag="wcol_e")
        sq = sb.tile([128, 1024], F32, tag="sq")
        ek = sb.tile([128, 1024], BF, tag="ek")
        kv = sb.tile([128, 1024], BF, tag="kv")
        ekp = sb.tile([128, 1024], BF, tag="ekp")
        kvp = sb.tile([128, 1024], BF, tag="kvp")
        rden = sb.tile([128, 1024], F32, tag="rden")
        w1 = sb.tile([128, 1024], F32, tag="w1")
        w2 = sb.tile([128, 1024], BF, tag="w2")
        warm = sb.tile([128, 512], BF, tag="warm")

        # input loads, in consumption-priority order
        nc.sync.dma_start(out=xt[:, 0:1024], in_=xt_d[:, 0:1024])
        nc.sync.dma_start(out=xt[:, 1024:2048], in_=xt_d[:, 1024:2048])
        nc.sync.dma_start(out=wkv[:], in_=wkv_d[:])
        nc.sync.dma_start(out=wq[:], in_=wq_d[:])
        nc.sync.dma_start(out=xt2[:, 0:1024], in_=xt2_d[:, 0:1024])
        nc.sync.dma_start(out=xt2[:, 1024:2048], in_=xt2_d[:, 1024:2048])
        nc.sync.dma_start(out=wrow_e[:], in_=wrow_d[:])
        nc.sync.dma_start(out=wcol_e[:], in_=wcol_d[:])

        # PE warm-up: dummy matmuls while the input DMAs stream in, so the
        # HAM clock gate is released (1.2 -> 2.4 GHz) before the real work
        nc.gpsimd.memset(warm[:], 0.0)
        for i in range(10):
            wps = ps_kv.tile([128, 512], F32, tag="kvps")
            nc.tensor.matmul(wps[:], warm[:, 0:128], warm[:], start=True,
                             stop=True)

        # zero the grid accumulators on the (idle-early) DVE; every grid
        # matmul then accumulates with start=False + skip_group_check
        numT = ps_g.tile([128, 1024], F32, tag="numT")
        denT = ps_g.tile([128, 1024], F32, tag="denT")
        for t_ in (numT, denT):
            for bank in range(2):
                nc.vector.memset(t_[:, bank * 512:(bank + 1) * 512], 0.0)

        # q projection, feature-major: qT[o, t] accumulated over 2 d-halves,
        # copied to SBUF right away (frees the PSUM bank; DVE is idle early)
        qsb = sb.tile([128, 1024], F32, tag="qsb")
        for nh in range(2):
            qp = ps_q.tile([128, 512], F32, tag="qp")
            for kh in range(2):
                nc.tensor.matmul(
                    qp[:],
                    wq[:, kh * 128:(kh + 1) * 128],
                    xt[:, kh * 1024 + nh * 512: kh * 1024 + nh * 512 + 512],
                    start=(kh == 0), stop=(kh == 1),
                )
            nc.vector.tensor_copy(qsb[:, nh * 512:(nh + 1) * 512], qp[:])

        # sigmoids early (from SBUF): ACT does Sigmoid once, then only Exp
        nc.scalar.activation(sq[:, 0:512], qsb[:, 0:512], AF.Sigmoid)
        nc.scalar.activation(sq[:, 512:1024], qsb[:, 512:1024], AF.Sigmoid)

        # k|v projections token-major; normal token order (xt) feeds the row
        # part, grid-transposed token order (xt2) feeds the col part. Two
        # token-slices share one PSUM tile so exp/mul run at [2,128] each.
        ek_vw = ek[:].rearrange("p (g f) -> p g f", f=128)
        kv_vw = kv[:].rearrange("p (g f) -> p g f", f=128)
        ekp_vw = ekp[:].rearrange("p (g f) -> p g f", f=128)
        kvp_vw = kvp[:].rearrange("p (g f) -> p g f", f=128)
        for src, ek_t, kv_t in ((xt, ek_vw, kv_vw), (xt2, ekp_vw, kvp_vw)):
            for pr in range(4):
                kvps = ps_kv.tile([128, 512], F32, tag="kvps")
                mm = {}
                for g2 in range(2):
                    g = 2 * pr + g2
                    for kh in range(2):
                        lhsT = src[:, kh * 1024 + g * 128: kh * 1024 + (g + 1) * 128]
                        mm[g2, kh] = nc.tensor.matmul(
                            kvps[:, g2 * 256:(g2 + 1) * 256],
                            lhsT,
                            wkv[:, kh * 256:(kh + 1) * 256],
                            start=(g2 == 0 and kh == 0),
                            stop=(g2 == 1 and kh == 1),
                        )
                # keep PSUM zero-region state machine ordering legal: the
                # start=True matmul first, the stop=True matmul last
                add_dep_helper(mm[1, 0].ins, mm[0, 0].ins, reason="psum start first")
                add_dep_helper(mm[1, 1].ins, mm[0, 1].ins, reason="psum stop last")
                kvps_v = kvps[:].rearrange("p (g c) -> p g c", g=2)
                ps = slice(2 * pr, 2 * pr + 2)
                nc.scalar.activation(ek_t[:, ps, :], kvps_v[:, :, 0:128], AF.Exp)
                nc.vector.tensor_mul(kv_t[:, ps, :], ek_t[:, ps, :],
                                     kvps_v[:, :, 128:256])

        # grid reduction: row part (contiguous out cols per 4-grid-row slice)
        GK = dict(start=False, stop=False, skip_group_check=True)
        for g in range(8):
            gs = slice(g * 128, (g + 1) * 128)
            nc.tensor.matmul(numT[:, gs], kv[:, gs], wrow_e[:, gs], **GK)
            nc.tensor.matmul(denT[:, gs], ek[:, gs], wrow_e[:, gs], **GK)

        # col part: out cols strided (token r*32+c); bank-major order so the
        # bank-0 combine below can start while bank-1 matmuls still run
        numT_v = numT[:].rearrange("p (r c) -> p c r", c=N)
        denT_v = denT[:].rearrange("p (r c) -> p c r", c=N)
        wcol_v = wcol_e[:].rearrange("p (g cb r) -> p g cb r", g=8, cb=4)
        for hb in range(2):
            rh = slice(hb * 16, (hb + 1) * 16)
            # den first: the combine's reciprocal (the longest chain) can
            # start as soon as this bank's den matmuls finish
            for g in range(8):
                gs = slice(g * 128, (g + 1) * 128)
                nc.tensor.matmul(denT_v[:, 4 * g:4 * (g + 1), rh],
                                 ekp[:, gs], wcol_v[:, g, :, rh], **GK)
            for g in range(8):
                gs = slice(g * 128, (g + 1) * 128)
                nc.tensor.matmul(numT_v[:, 4 * g:4 * (g + 1), rh],
                                 kvp[:, gs], wcol_v[:, g, :, rh], **GK)

        # combine in quarters: the trailing quarter's chain (recip -> 2 muls
        # -> DMA) is all that sits past the last col matmul. The SBUF-only
        # final multiply runs on the idle Pool engine for the first three
        # quarters so DVE keeps feeding reciprocals.
        for qt in range(4):
            hs = slice(qt * 256, (qt + 1) * 256)
            nc.vector.reciprocal_approx_fast(rden[:, hs], denT[:, hs])
            nc.vector.tensor_mul(w1[:, hs], rden[:, hs], numT[:, hs])
            eng = nc.vector if qt == 3 else nc.gpsimd
            eng.tensor_mul(w2[:, hs], w1[:, hs], sq[:, hs])
            nc.sync.dma_start(out=out_d[:, hs], in_=w2[:, hs])

    nc.compile()
    return nc


def _get_nc():
    global _CACHED_NC
    if _CACHED_NC is None:
        _CACHED_NC = _build_nc()
    return _CACHED_NC


def _interleave_halves(a):
    """[256, M] -> [128, 2*M] with cols (half, m); partitions = dim%128."""
    return np.concatenate([a[0:128], a[128:256]], axis=1)


def make_shards(x, Wq, Wk, Wv, wbias):
    """Build the per-core input maps (host-side layout/sharding only)."""
    bf = ml_dtypes.bfloat16
    B = x.shape[0]

    # neighbor-weight reorganization: for token t=(r,c), sorted wbias cols are
    #   [0, r)   -> col-neighbor grid-row j = pos
    #   [r, r+N) -> row-neighbor grid-col j = pos - r
    #   [r+N, 2N-1) -> col-neighbor grid-row j = pos - (N - 1)
    t_idx = np.arange(T)
    r_of = t_idx // N
    Wr = np.empty((T, N), np.float32)
    Wc = np.full((T, N), NEG, np.float32)
    for t in range(T):
        r = int(r_of[t])
        Wr[t] = wbias[t, r:r + N]
        Wc[t, :r] = wbias[t, :r]
        Wc[t, r + 1:] = wbias[t, r + N:]

    # block-diagonal row weights (exp-folded; off-diagonal exp(NEG) == 0):
    #   wrow[rb*32+j, g*128 + rb*32 + c] = exp(Wr[(4g+rb)*32 + c, j])
    wrow = np.full((128, 1024), NEG, np.float32)
    wcol = np.full((128, 1024), NEG, np.float32)
    rb, j, c = np.meshgrid(np.arange(4), np.arange(N), np.arange(N),
                           indexing="ij")
    for g in range(8):
        wrow[rb * N + j, g * 128 + rb * N + c] = Wr[(4 * g + rb) * N + c, j]
        # col weights: wcol[cb*32+j, g*128 + cb*32 + r] = Wc[r*32 + 4g+cb, j]
        cb, jj, rr = rb, j, c
        wcol[cb * N + jj, g * 128 + cb * N + rr] = Wc[rr * N + 4 * g + cb, jj]
    wrow = np.exp(wrow).astype(bf)
    wcol = np.exp(wcol).astype(bf)

    xt_b = []
    xt2_b = []
    for b in range(B):
        xtb = np.ascontiguousarray(_interleave_halves(x[b].T)).astype(bf)
        xt_b.append(xtb)
        # grid-transposed token order: col kh*1024 + c*32 + r <- kh*1024 + r*32 + c
        x2 = xtb.reshape(128, 2, N, N).transpose(0, 1, 3, 2).reshape(128, 2048)
        xt2_b.append(np.ascontiguousarray(x2))

    in_maps = []
    for core in range(8):
        b, hp = core // 2, core % 2
        sl = slice(hp * 128, (hp + 1) * 128)
        wq_c = _interleave_halves(Wq[sl].T).astype(bf)                # [128,256]
        k_h = Wk[sl].T.reshape(2, 128, 128)
        v_h = Wv[sl].T.reshape(2, 128, 128)
        wkv_c = np.concatenate([k_h[0], v_h[0], k_h[1], v_h[1]],
                               axis=1).astype(bf)                      # [128,512]
        in_maps.append({
            "xt": xt_b[b],
            "xt2": xt2_b[b],
            "wq": np.ascontiguousarray(wq_c),
            "wkv": np.ascontiguousarray(wkv_c),
            "wrow": wrow,
            "wcol": wcol,
        })
    return in_maps


def kernel(x, Wq, Wk, Wv, wbias, key_indices=None, **_unused):
    global LAST_RESULT
    x = np.asarray(x, np.float32)
    Wq = np.asarray(Wq, np.float32)
    Wk = np.asarray(Wk, np.float32)
    Wv = np.asarray(Wv, np.float32)
    wbias = np.asarray(wbias, np.float32)

    nc = _get_nc()
    in_maps = make_shards(x, Wq, Wk, Wv, wbias)
    try:
        res = run_bass_kernel_spmd(nc, in_maps, core_ids=list(range(8)))
    except ModuleNotFoundError:
        # BASS_TRACE set but the NTFF profile hook module is unavailable in
        # this environment -- rerun untraced
        os.environ["BASS_NEVER_TRACE"] = "1"
        res = run_bass_kernel_spmd(nc, in_maps, core_ids=list(range(8)))
    LAST_RESULT = res

    B = x.shape[0]
    out = np.empty((B, T, D), np.float32)
    for core in range(8):
        b, hp = core // 2, core % 2
        out[b, :, hp * 128:(hp + 1) * 128] = \
            res.results[core]["out"].astype(np.float32).T
    return out
